# revision 1
# baseline (speedup 1.0000x reference)
"""MultiHeadAttention Trainium2 kernel.

Sharding: 8 cores = 4 batches x 2 head-halves. Core c handles batch c//2,
heads (c%2)*8 .. +8. Each core computes a partial output projection
(its 8 heads' contribution, 512 of the 1024 contraction dims of Wo);
the host sums the two partials per batch and adds the bias.

Per-core dataflow (S=2048 seq, D=1024 model, 8 local heads of 64):
  phase 1: PE-transpose each input X (query/key/value) tile-by-tile to get
           X^T, then project:
             q^T (zero-padded per head to 128 partitions, for head-pair
                  packed score matmuls), k^T (head-pair layout), and
             v_aug (v chunks with a ones column appended -> softmax
                  denominator comes free out of the ctx matmul).
  phase 2: per (q-tile of 512, head): scores^T = k^T_pair.T @ q_pad
           (PSUM), exp on ACT straight PSUM->SBUF (scale=1/sqrt(D) folded
           into the activation), ctx^T[65,512] = [v|1]^T @ E accumulated
           over 16 key chunks; row 64 is the softmax denominator.
           Normalize: 1/D = exp(-ln(D)) on ACT (ln+exp share one table
           set), broadcast across partitions via a ones[1,128] PE matmul,
           then one DVE multiply per head. Output projection per q-tile.
All matmuls run as float32r (~1.4e-4 rel err, 4x faster than fp32 mode).
"""

import os
import sys

# The bass kernel needs the TRN (axon) jax backend; if the caller pinned
# JAX_PLATFORMS=cpu for its reference computation, undo that before jax
# gets imported (no-op if jax is already initialized).
if os.environ.get("JAX_PLATFORMS") == "cpu":
    del os.environ["JAX_PLATFORMS"]

sys.path.insert(0, "/opt/trn_rl_repo")

import numpy as np
import ml_dtypes

import concourse.bass as bass
import concourse.mybir as mybir
import concourse.tile as tile
from concourse import bacc
from concourse.bass_utils import run_bass_kernel_spmd
from concourse.masks import make_identity
import concourse.hw_specs as hw_specs

_orig_get_act_tables = hw_specs.get_activation_tables


def _only_ln_exp_set(module_arch):
    # Keep all set names in original order (set_id = dict index), but leave
    # only natural_log_exp_and_others non-empty so every activation resolves
    # to that one set -> a single ACT_TABLE_LOAD for exp+ln+copy.
    t = _orig_get_act_tables(module_arch)
    name = "natural_log_exp_and_others"
    assert name in t, list(t)
    return {k: (v if k == name else set()) for k, v in t.items()}


hw_specs.get_activation_tables = _only_ln_exp_set
bacc.get_activation_tables = _only_ln_exp_set

f32 = mybir.dt.float32
f32r = mybir.dt.float32r
P = 128
S = 2048
D = 1024
HLOC = 8          # heads per core
HK = 64           # head size
DKL = HLOC * HK   # local projected dims = 512
QT = 512          # q-tile size
NQT = S // QT     # 4
NKC = S // P      # 16 key chunks
EXP = mybir.ActivationFunctionType.Exp
LN = mybir.ActivationFunctionType.Ln
SCALE = 1.0 / np.sqrt(np.float32(D))  # = 1/32, exact


def _round_f32r(a):
    hi = a.astype(ml_dtypes.bfloat16).astype(np.float32)
    lo = (a - hi).astype(ml_dtypes.bfloat16).astype(np.float32)
    return hi + lo


def build_nc():
    nc = bacc.Bacc("TRN2", target_bir_lowering=False)
    xq = nc.dram_tensor("xq", [S, D], f32, kind="ExternalInput")
    xk = nc.dram_tensor("xk", [S, D], f32, kind="ExternalInput")
    xv = nc.dram_tensor("xv", [S, D], f32, kind="ExternalInput")
    wq = nc.dram_tensor("wq", [D, DKL], f32r, kind="ExternalInput")
    wk = nc.dram_tensor("wk", [D, DKL], f32r, kind="ExternalInput")
    wv = nc.dram_tensor("wv", [D, DKL], f32r, kind="ExternalInput")
    wot = nc.dram_tensor("wot", [DKL, D], f32r, kind="ExternalInput")
    out = nc.dram_tensor("out", [S, D], f32, kind="ExternalOutput")

    ds, ts = bass.ds, bass.ts

    with tile.TileContext(nc) as tc:
        with tc.tile_pool(name="persist", bufs=1) as pp:
            # head h's q^T, zero-padded to the 128-partition pair layout
            qpad = pp.tile([P, HLOC, S], f32r, name="qpad")
            # k^T in head-pair layout: [:, p, :] rows 0-63 = head 2p, 64-127 = 2p+1
            kT = pp.tile([P, HLOC // 2, S], f32r, name="kT")
            # v chunks + ones column: [k-part, chunk, head, 64 v | 1]
            vaug = pp.tile([P, NKC, HLOC, HK + 1], f32r, name="vaug")
            idn = pp.tile([P, P], f32, name="idn")
            ones1 = pp.tile([1, P], f32r, name="ones1")
            nc.vector.memset(ones1[:].bitcast(f32), 1.0)

            make_identity(nc, idn[:])
            nc.vector.memset(qpad[:].bitcast(f32), 0.0)
            nc.vector.memset(vaug[:].bitcast(f32), 1.0)

            # ---------------- phase 1: transpose + projections ----------------
            with tc.tile_pool(name="p1sb", bufs=1) as p1, tc.tile_pool(
                name="tp_ps", bufs=3, space="PSUM"
            ) as tp_ps, tc.tile_pool(name="pr_ps", bufs=3, space="PSUM") as pr_ps:
                for t, (xdram, wdram) in enumerate(((xq, wq), (xk, wk), (xv, wv))):
                    w_sb = p1.tile([P, D // P, DKL], f32r, name=f"w{t}", tag="w", bufs=2)
                    nc.sync.dma_start(
                        w_sb[:], wdram.rearrange("(dc p) n -> p dc n", p=P)
                    )
                    for st in range(S // QT):
                        # stage 4 row-blocks of X
                        stg = []
                        for sb in range(4):
                            xst = p1.tile([P, D], f32, name="xst", tag="xst", bufs=5)
                            nc.sync.dma_start(
                                xst[:], xdram[ds(st * QT + sb * P, P), :]
                            )
                            stg.append(xst)
                        # transpose to X^T tile [128 d, dchunk, 512 s]
                        xt = p1.tile([P, D // P, QT], f32r, name="xt", tag="xt")
                        for dc in range(D // P):
                            ptp = tp_ps.tile([P, QT], f32, name="ptp")
                            for sb in range(4):
                                nc.tensor.transpose(
                                    ptp[:, ds(sb * P, P)],
                                    stg[sb][:, ds(dc * P, P)],
                                    idn[:],
                                )
                            nc.scalar.copy(xt[:, dc, :], ptp[:])
                        if t < 2:
                            # q^T / k^T orientation: psum [128 dk, 512 s]
                            for dkb in range(DKL // P):
                                ppr = pr_ps.tile([P, QT], f32, name="ppr")
                                for dc in range(D // P):
                                    nc.tensor.matmul(
                                        ppr[:],
                                        w_sb[:, dc, ds(dkb * P, P)],
                                        xt[:, dc, :],
                                        start=(dc == 0),
                                        stop=(dc == D // P - 1),
                                    )
                                if t == 0:
                                    nc.vector.tensor_copy(
                                        qpad[0:64, 2 * dkb, ds(st * QT, QT)],
                                        ppr[0:64, :],
                                    )
                                    nc.vector.tensor_copy(
                                        qpad[64:128, 2 * dkb + 1, ds(st * QT, QT)],
                                        ppr[64:128, :],
                                    )
                                else:
                                    nc.vector.tensor_copy(
                                        kT[:, dkb, ds(st * QT, QT)], ppr[:]
                                    )
                        else:
                            # v orientation: psum [128 s, 512 dk]
                            for sb in range(4):
                                ppr = pr_ps.tile([P, DKL], f32, name="ppr")
                                for dc in range(D // P):
                                    nc.tensor.matmul(
                                        ppr[:],
                                        xt[:, dc, ds(sb * P, P)],
                                        w_sb[:, dc, :],
                                        start=(dc == 0),
                                        stop=(dc == D // P - 1),
                                    )
                                ci = st * 4 + sb
                                nc.vector.tensor_copy(
                                    vaug[:, ci, :, 0:HK],
                                    ppr[:].rearrange("p (h k) -> p h k", k=HK),
                                )

            # ---------------- phase 2: attention + output projection ----------
            with tc.tile_pool(name="p2sb", bufs=1) as p2, tc.tile_pool(
                name="et_pool", bufs=3
            ) as etp, tc.tile_pool(name="bc_pool", bufs=2) as bcp, tc.tile_pool(
                name="cu_pool", bufs=1
            ) as cup, tc.tile_pool(name="ct_pool", bufs=2) as ctp, tc.tile_pool(
                name="ob_pool", bufs=2
            ) as obp, tc.tile_pool(
                name="sc_ps", bufs=2, space="PSUM"
            ) as sc_ps, tc.tile_pool(
                name="ct_ps", bufs=1, space="PSUM"
            ) as ct_ps, tc.tile_pool(
                name="op_ps", bufs=1, space="PSUM"
            ) as op_ps, tc.tile_pool(
                name="bc_ps", bufs=2, space="PSUM"
            ) as bc_ps:
                wot_sb = p2.tile([P, DKL // P, D], f32r, name="wot_sb")
                nc.sync.dma_start(
                    wot_sb[:], wot.rearrange("(c p) n -> p c n", p=P)
                )
                for qt in range(NQT):
                    ctxU = cup.tile([P, DKL // P, QT], f32, name="ctxU")
                    ctxT = ctp.tile([P, DKL // P, QT], f32r, name="ctxT")
                    for h in range(HLOC):
                        p = h // 2
                        pct = ct_ps.tile([HK + 1, QT], f32, name="pct")
                        for g in range(NKC // 2):
                            psc = sc_ps.tile([P, 2, QT], f32, name="psc")
                            for j in range(2):
                                ch = 2 * g + j
                                nc.tensor.matmul(
                                    psc[:, j, :],
                                    kT[:, p, ds(ch * P, P)],
                                    qpad[:, h, ds(qt * QT, QT)],
                                    start=True,
                                    stop=True,
                                )
                            ete = etp.tile([P, 2, QT], f32r, name="ete")
                            nc.scalar.activation(ete[:], psc[:], EXP, scale=SCALE)
                            for j in range(2):
                                ch = 2 * g + j
                                nc.tensor.matmul(
                                    pct[:],
                                    vaug[:, ch, h, :],
                                    ete[:, j, :],
                                    start=(ch == 0),
                                    stop=(ch == NKC - 1),
                                )
                        sl = slice(64 * (h % 2), 64 * (h % 2) + 64)
                        nc.vector.tensor_copy(ctxU[sl, h // 2, :], pct[0:64, :])
                        dln = bcp.tile([1, QT], f32, name="dln", tag="dln", bufs=3)
                        nc.scalar.activation(dln[:], pct[64:65, :], LN)
                        drec = bcp.tile([1, QT], f32r, name="drec", tag="drec", bufs=3)
                        nc.scalar.activation(drec[:], dln[:], EXP, scale=-1.0)
                        bch = bc_ps.tile([P, QT], f32, name="bch")
                        nc.tensor.matmul(bch[:], ones1[:], drec[:], start=True, stop=True)
                        nc.vector.tensor_mul(
                            out=ctxT[sl, h // 2, :],
                            in0=ctxU[sl, h // 2, :],
                            in1=bch[sl, :],
                        )
                    for sb in range(QT // P):
                        ob = obp.tile([P, 2, QT], f32, name="ob")
                        for nh in range(2):
                            pop = op_ps.tile([P, QT], f32, name="pop")
                            for cc in range(DKL // P):
                                nc.tensor.matmul(
                                    pop[:],
                                    ctxT[:, cc, ds(sb * P, P)],
                                    wot_sb[:, cc, ds(nh * QT, QT)],
                                    start=(cc == 0),
                                    stop=(cc == DKL // P - 1),
                                )
                            nc.vector.tensor_copy(ob[:, nh, :], pop[:])
                        nc.sync.dma_start(
                            out[ds(qt * QT + sb * P, P), :],
                            ob[:].rearrange("p a b -> p (a b)"),
                        )
    nc.compile()
    return nc


_NC_CACHE = []


def _get_nc():
    if not _NC_CACHE:
        _NC_CACHE.append(build_nc())
    return _NC_CACHE[0]


def make_in_maps(query, key_in, value, Wq, Wk, Wv, Wo, bo):
    query = np.asarray(query, dtype=np.float32)
    key_in = np.asarray(key_in, dtype=np.float32)
    value = np.asarray(value, dtype=np.float32)
    Wq = np.asarray(Wq, dtype=np.float32)
    Wk = np.asarray(Wk, dtype=np.float32)
    Wv = np.asarray(Wv, dtype=np.float32)
    Wo = np.asarray(Wo, dtype=np.float32)
    in_maps = []
    for c in range(8):
        b = c // 2
        h0 = (c % 2) * HLOC
        wq_c = _round_f32r(
            np.ascontiguousarray(
                Wq[h0 : h0 + HLOC].transpose(1, 0, 2).reshape(D, DKL)
            )
        )
        wk_c = _round_f32r(
            np.ascontiguousarray(
                Wk[h0 : h0 + HLOC].transpose(1, 0, 2).reshape(D, DKL)
            )
        )
        wv_c = _round_f32r(
            np.ascontiguousarray(
                Wv[h0 : h0 + HLOC].transpose(1, 0, 2).reshape(D, DKL)
            )
        )
        wot_c = _round_f32r(
            np.ascontiguousarray(Wo[:, h0 * HK : h0 * HK + DKL].T)
        )
        in_maps.append(
            {
                "xq": np.ascontiguousarray(query[b]),
                "xk": np.ascontiguousarray(key_in[b]),
                "xv": np.ascontiguousarray(value[b]),
                "wq": wq_c,
                "wk": wk_c,
                "wv": wv_c,
                "wot": wot_c,
            }
        )
    return in_maps


def kernel(query, key_in, value, Wq, Wk, Wv, Wo, bo):
    nc = _get_nc()
    in_maps = make_in_maps(query, key_in, value, Wq, Wk, Wv, Wo, bo)
    res = run_bass_kernel_spmd(nc, in_maps, list(range(8)))
    bo = np.asarray(bo, dtype=np.float32)
    B = np.asarray(query).shape[0]
    out = np.empty((B, S, D), dtype=np.float32)
    for b in range(B):
        out[b] = res.results[2 * b]["out"] + res.results[2 * b + 1]["out"] + bo
    return out


if __name__ == "__main__":
    rng = np.random.default_rng(0)
    q = rng.standard_normal((4, S, D), dtype=np.float32)
    k = rng.standard_normal((4, S, D), dtype=np.float32)
    v = rng.standard_normal((4, S, D), dtype=np.float32)
    sd = 1.0 / np.sqrt(D)
    Wq = rng.standard_normal((16, D, HK), dtype=np.float32) * sd
    Wk = rng.standard_normal((16, D, HK), dtype=np.float32) * sd
    Wv = rng.standard_normal((16, D, HK), dtype=np.float32) * sd
    Wo = rng.standard_normal((D, D), dtype=np.float32) * sd
    bo = rng.standard_normal((D,), dtype=np.float32) * 0.01
    o = kernel(q, k, v, Wq, Wk, Wv, Wo, bo)
    print("out", o.shape, o.dtype, np.abs(o).max())



# revision 4
# speedup vs baseline: 3.3090x; 3.3090x over previous
"""MultiHeadAttention Trainium2 kernel.

Sharding: 8 cores = 4 batches x 2 head-halves. Core c handles batch c//2,
heads (c%2)*8 .. +8. I/O is minimized (the axon per-call cost is dominated
by host->device bytes): each core receives ONE packed bf16 input tensor
holding only its seq-half of X^T (q/k/v, pre-transposed on host) plus a
quarter shard of its head-half weights. On device, a pair AllGather
rebuilds full X^T per batch, a 4-way AllGather rebuilds the weights, and
a pair ReduceScatter sums the two head-half output projections so each
core returns only half the batch output rows (bf16).

Packed input xin [3584, 1024] per core (b = c//2, half = c%2):
  rows    0:1024  q^T of batch b, seq rows half*1024..+1024  [d, s_half]
  rows 1024:2048  k^T   (same slice)
  rows 2048:3072  v^T   (same slice)
  rows 3072:3584  weight quarter pack: for each of (wq, wk, wv, wot) in
                  [128 part, 4096] layout, partitions 32b..32b+32,
                  re-flowed to 128 rows of 1024.

Per-core dataflow (S=2048 seq, D=1024 model, 8 local heads of 64):
  phase 1: project q^T (zero-padded per head to 128 partitions for
           head-pair packed score matmuls), k^T (head-pair layout), and
           v_aug (v chunks with a ones column -> softmax denominator
           comes free out of the ctx matmul). All matmuls bf16.
  phase 2: per (q-tile of 512, head): scores^T = k^T_pair.T @ q_pad
           (PSUM f32), exp on ACT straight PSUM->SBUF bf16 (scale
           1/sqrt(D) folded in), ctx^T[65,512] = [v|1]^T @ E accumulated
           over 16 key chunks; row 64 is the softmax denominator.
           Normalize via 1/d = exp(-ln(d)) on ACT, broadcast across
           partitions with a ones[1,128] PE matmul, one DVE multiply per
           head. Output projection per q-tile -> partial out (bf16) ->
           ReduceScatter(add) over the batch pair.
"""

import os
import sys

# The bass kernel needs the TRN (axon) jax backend; if the caller pinned
# JAX_PLATFORMS=cpu for its reference computation, undo that before jax
# gets imported (no-op if jax is already initialized).
if os.environ.get("JAX_PLATFORMS") == "cpu":
    del os.environ["JAX_PLATFORMS"]

sys.path.insert(0, "/opt/trn_rl_repo")

import numpy as np
import ml_dtypes

import concourse.bass as bass
import concourse.mybir as mybir
import concourse.tile as tile
from concourse import bacc
from concourse.bass_utils import run_bass_kernel_spmd
import concourse.hw_specs as hw_specs

_orig_get_act_tables = hw_specs.get_activation_tables


def _only_ln_exp_set(module_arch):
    # Keep all set names in original order (set_id = dict index), but leave
    # only natural_log_exp_and_others non-empty so every activation resolves
    # to that one set -> a single ACT_TABLE_LOAD for exp+ln+copy.
    t = _orig_get_act_tables(module_arch)
    name = "natural_log_exp_and_others"
    assert name in t, list(t)
    return {k: (v if k == name else set()) for k, v in t.items()}


hw_specs.get_activation_tables = _only_ln_exp_set
bacc.get_activation_tables = _only_ln_exp_set

f32 = mybir.dt.float32
bf16 = mybir.dt.bfloat16
P = 128
S = 2048
D = 1024
SH = S // 2       # per-core seq half
HLOC = 8          # heads per core
HK = 64           # head size
DKL = HLOC * HK   # local projected dims = 512
QT = 512          # q-tile size
NQT = S // QT     # 4
NKC = S // P      # 16 key chunks
XROWS = 3 * D     # 3072 packed X^T rows
WROWS = 512       # packed weight-quarter rows
EXP = mybir.ActivationFunctionType.Exp
LN = mybir.ActivationFunctionType.Ln
SCALE = 1.0 / np.sqrt(np.float32(D))  # = 1/32, exact

PAIRS = [[0, 1], [2, 3], [4, 5], [6, 7]]
QUADS = [[0, 2, 4, 6], [1, 3, 5, 7]]


def build_nc():
    nc = bacc.Bacc("TRN2", target_bir_lowering=False)
    xin = nc.dram_tensor("xin", [XROWS + WROWS, SH], bf16, kind="ExternalInput")
    out = nc.dram_tensor("out", [SH, D], bf16, kind="ExternalOutput")

    ds = bass.ds

    with tile.TileContext(nc) as tc:
        with tc.tile_pool(name="dram", bufs=1, space="DRAM") as dram, tc.tile_pool(
            name="persist", bufs=1
        ) as pp:
            # collective bounce/result buffers (collectives can't touch I/O
            # tensors; gather outputs live in Shared scratchpad)
            xb = dram.tile([XROWS, SH], bf16, name="xb")
            wb = dram.tile([WROWS, SH], bf16, name="wb")
            xg = dram.tile([2, XROWS, SH], bf16, name="xg")
            wg = dram.tile([512, 4096], bf16, name="wg")
            po = dram.tile([S, D], bf16, name="po")
            og = dram.tile([SH, D], bf16, name="og")

            nc.gpsimd.dma_start(xb[:], xin[ds(0, XROWS), :])
            nc.gpsimd.dma_start(wb[:], xin[ds(XROWS, WROWS), :])
            nc.gpsimd.collective_compute(
                "AllGather",
                mybir.AluOpType.bypass,
                replica_groups=PAIRS,
                ins=[xb.opt()],
                outs=[xg.opt()],
            )
            nc.gpsimd.collective_compute(
                "AllGather",
                mybir.AluOpType.bypass,
                replica_groups=QUADS,
                ins=[wb.opt()],
                outs=[wg.opt()],
            )

            # head h's q^T, zero-padded to the 128-partition pair layout
            qpad = pp.tile([P, HLOC, S], bf16, name="qpad")
            # k^T in head-pair layout: [:, p, :] rows 0-63 = head 2p, 64-127 = 2p+1
            kT = pp.tile([P, HLOC // 2, S], bf16, name="kT")
            # v chunks + ones column: [k-part, chunk, head, 64 v | 1]
            vaug = pp.tile([P, NKC, HLOC, HK + 1], bf16, name="vaug")
            ones1 = pp.tile([1, P], bf16, name="ones1")
            nc.vector.memset(ones1[:], 1.0)
            nc.vector.memset(qpad[:], 0.0)
            nc.vector.memset(vaug[:], 1.0)

            # weights in SBUF: [128 part, (dc|cc)*free] flattened layouts
            wqs = pp.tile([P, 4096], bf16, name="wqs")
            wks = pp.tile([P, 4096], bf16, name="wks")
            wvs = pp.tile([P, 4096], bf16, name="wvs")
            wos = pp.tile([P, 4096], bf16, name="wos")
            for k in range(4):
                for i, wt in enumerate((wqs, wks, wvs, wos)):
                    nc.sync.dma_start(
                        wt[ds(32 * k, 32), :], wg[ds(128 * k + 32 * i, 32), :]
                    )

            # ---------------- phase 1: projections (X^T arrives gathered) ----
            with tc.tile_pool(name="p1sb", bufs=1) as p1, tc.tile_pool(
                name="pr_ps", bufs=3, space="PSUM"
            ) as pr_ps:
                for t, ws in enumerate((wqs, wks, wvs)):
                    toff = t * D
                    for st in range(S // QT):
                        r, sloc = st // 2, (st % 2) * QT
                        xt = p1.tile([P, D // P, QT], bf16, name="xt", tag="xt", bufs=3)
                        for dc in range(D // P):
                            nc.sync.dma_start(
                                xt[:, dc, :],
                                xg[r, ds(toff + dc * P, P), ds(sloc, QT)],
                            )
                        if t < 2:
                            # q^T / k^T orientation: psum [128 dk, 512 s]
                            for dkb in range(DKL // P):
                                ppr = pr_ps.tile([P, QT], f32, name="ppr")
                                for dc in range(D // P):
                                    nc.tensor.matmul(
                                        ppr[:],
                                        ws[:, ds(dc * DKL + dkb * P, P)],
                                        xt[:, dc, :],
                                        start=(dc == 0),
                                        stop=(dc == D // P - 1),
                                    )
                                if t == 0:
                                    nc.vector.tensor_copy(
                                        qpad[0:64, 2 * dkb, ds(st * QT, QT)],
                                        ppr[0:64, :],
                                    )
                                    nc.vector.tensor_copy(
                                        qpad[64:128, 2 * dkb + 1, ds(st * QT, QT)],
                                        ppr[64:128, :],
                                    )
                                else:
                                    nc.vector.tensor_copy(
                                        kT[:, dkb, ds(st * QT, QT)], ppr[:]
                                    )
                        else:
                            # v orientation: psum [128 s, 512 dk]
                            for sb in range(4):
                                ppr = pr_ps.tile([P, DKL], f32, name="ppr")
                                for dc in range(D // P):
                                    nc.tensor.matmul(
                                        ppr[:],
                                        xt[:, dc, ds(sb * P, P)],
                                        ws[:, ds(dc * DKL, DKL)],
                                        start=(dc == 0),
                                        stop=(dc == D // P - 1),
                                    )
                                ci = st * 4 + sb
                                nc.vector.tensor_copy(
                                    vaug[:, ci, :, 0:HK],
                                    ppr[:].rearrange("p (h k) -> p h k", k=HK),
                                )

            # ---------------- phase 2: attention + output projection ----------
            with tc.tile_pool(name="et_pool", bufs=3) as etp, tc.tile_pool(
                name="bc_pool", bufs=2
            ) as bcp, tc.tile_pool(name="cu_pool", bufs=1) as cup, tc.tile_pool(
                name="ct_pool", bufs=2
            ) as ctp, tc.tile_pool(
                name="ob_pool", bufs=2
            ) as obp, tc.tile_pool(
                name="sc_ps", bufs=2, space="PSUM"
            ) as sc_ps, tc.tile_pool(
                name="ct_ps", bufs=1, space="PSUM"
            ) as ct_ps, tc.tile_pool(
                name="op_ps", bufs=1, space="PSUM"
            ) as op_ps, tc.tile_pool(
                name="bc_ps", bufs=2, space="PSUM"
            ) as bc_ps:
                for qt in range(NQT):
                    ctxU = cup.tile([P, DKL // P, QT], f32, name="ctxU")
                    ctxT = ctp.tile([P, DKL // P, QT], bf16, name="ctxT")
                    for h in range(HLOC):
                        p = h // 2
                        pct = ct_ps.tile([HK + 1, QT], f32, name="pct")
                        for g in range(NKC // 2):
                            psc = sc_ps.tile([P, 2, QT], f32, name="psc")
                            for j in range(2):
                                ch = 2 * g + j
                                nc.tensor.matmul(
                                    psc[:, j, :],
                                    kT[:, p, ds(ch * P, P)],
                                    qpad[:, h, ds(qt * QT, QT)],
                                    start=True,
                                    stop=True,
                                )
                            ete = etp.tile([P, 2, QT], bf16, name="ete")
                            nc.scalar.activation(ete[:], psc[:], EXP, scale=SCALE)
                            for j in range(2):
                                ch = 2 * g + j
                                nc.tensor.matmul(
                                    pct[:],
                                    vaug[:, ch, h, :],
                                    ete[:, j, :],
                                    start=(ch == 0),
                                    stop=(ch == NKC - 1),
                                )
                        sl = slice(64 * (h % 2), 64 * (h % 2) + 64)
                        nc.vector.tensor_copy(ctxU[sl, h // 2, :], pct[0:64, :])
                        dln = bcp.tile([1, QT], f32, name="dln", tag="dln", bufs=3)
                        nc.scalar.activation(dln[:], pct[64:65, :], LN)
                        drec = bcp.tile([1, QT], bf16, name="drec", tag="drec", bufs=3)
                        nc.scalar.activation(drec[:], dln[:], EXP, scale=-1.0)
                        bch = bc_ps.tile([P, QT], f32, name="bch")
                        nc.tensor.matmul(bch[:], ones1[:], drec[:], start=True, stop=True)
                        nc.vector.tensor_mul(
                            out=ctxT[sl, h // 2, :],
                            in0=ctxU[sl, h // 2, :],
                            in1=bch[sl, :],
                        )
                    for sb in range(QT // P):
                        ob = obp.tile([P, 2, QT], bf16, name="ob")
                        for nh in range(2):
                            pop = op_ps.tile([P, QT], f32, name="pop")
                            for cc in range(DKL // P):
                                nc.tensor.matmul(
                                    pop[:],
                                    ctxT[:, cc, ds(sb * P, P)],
                                    wos[:, ds(cc * D + nh * QT, QT)],
                                    start=(cc == 0),
                                    stop=(cc == DKL // P - 1),
                                )
                            nc.vector.tensor_copy(ob[:, nh, :], pop[:])
                        nc.sync.dma_start(
                            po[ds(qt * QT + sb * P, P), :],
                            ob[:].rearrange("p a b -> p (a b)"),
                        )

            nc.gpsimd.collective_compute(
                "ReduceScatter",
                mybir.AluOpType.add,
                replica_groups=PAIRS,
                ins=[po.opt()],
                outs=[og.opt()],
            )
            nc.gpsimd.dma_start(out[:, :], og[:])
    nc.compile()
    return nc


_NC_CACHE = []


def _get_nc():
    if not _NC_CACHE:
        _NC_CACHE.append(build_nc())
    return _NC_CACHE[0]


def _pack_w(w_hh, quarter):
    # [1024, 512] -> [128 part, 4096] (p, dc*512+n) -> quarter [32, 4096]
    # re-flowed to [128, 1024] rows of the packed input.
    a = w_hh.reshape(8, 128, 512).transpose(1, 0, 2).reshape(128, 4096)
    return a[32 * quarter : 32 * quarter + 32].reshape(128, 1024)


def _pack_wot(wot_c, quarter):
    # [512, 1024] -> [128 part, 4096] (p, cc*1024+n) -> quarter as above
    a = wot_c.reshape(4, 128, 1024).transpose(1, 0, 2).reshape(128, 4096)
    return a[32 * quarter : 32 * quarter + 32].reshape(128, 1024)


def make_in_maps(query, key_in, value, Wq, Wk, Wv, Wo, bo):
    query = np.asarray(query, dtype=np.float32)
    key_in = np.asarray(key_in, dtype=np.float32)
    value = np.asarray(value, dtype=np.float32)
    Wq = np.asarray(Wq, dtype=np.float32)
    Wk = np.asarray(Wk, dtype=np.float32)
    Wv = np.asarray(Wv, dtype=np.float32)
    Wo = np.asarray(Wo, dtype=np.float32)
    in_maps = []
    for c in range(8):
        b, half = c // 2, c % 2
        h0 = half * HLOC
        xin = np.empty((XROWS + WROWS, SH), dtype=ml_dtypes.bfloat16)
        rows = slice(half * SH, (half + 1) * SH)
        xin[0:D] = query[b, rows, :].T
        xin[D : 2 * D] = key_in[b, rows, :].T
        xin[2 * D : 3 * D] = value[b, rows, :].T
        wq_hh = Wq[h0 : h0 + HLOC].transpose(1, 0, 2).reshape(D, DKL)
        wk_hh = Wk[h0 : h0 + HLOC].transpose(1, 0, 2).reshape(D, DKL)
        wv_hh = Wv[h0 : h0 + HLOC].transpose(1, 0, 2).reshape(D, DKL)
        wot_c = np.ascontiguousarray(Wo[:, h0 * HK : h0 * HK + DKL].T)
        xin[XROWS + 0 : XROWS + 128] = _pack_w(wq_hh, b)
        xin[XROWS + 128 : XROWS + 256] = _pack_w(wk_hh, b)
        xin[XROWS + 256 : XROWS + 384] = _pack_w(wv_hh, b)
        xin[XROWS + 384 : XROWS + 512] = _pack_wot(wot_c, b)
        in_maps.append({"xin": xin})
    return in_maps


def gather_out(results, B, bo):
    """results: list of 8 per-core dicts with 'out' [SH, D] bf16."""
    bo = np.asarray(bo, dtype=np.float32)
    out = np.empty((B, S, D), dtype=np.float32)
    for b in range(B):
        out[b, 0:SH] = results[2 * b]["out"]
        out[b, SH:S] = results[2 * b + 1]["out"]
    out += bo
    return out


def kernel(query, key_in, value, Wq, Wk, Wv, Wo, bo):
    nc = _get_nc()
    in_maps = make_in_maps(query, key_in, value, Wq, Wk, Wv, Wo, bo)
    res = run_bass_kernel_spmd(nc, in_maps, list(range(8)))
    return gather_out(res.results, np.asarray(query).shape[0], bo)


if __name__ == "__main__":
    rng = np.random.default_rng(0)
    q = rng.standard_normal((4, S, D), dtype=np.float32)
    k = rng.standard_normal((4, S, D), dtype=np.float32)
    v = rng.standard_normal((4, S, D), dtype=np.float32)
    sd = 1.0 / np.sqrt(D)
    Wq = rng.standard_normal((16, D, HK), dtype=np.float32) * sd
    Wk = rng.standard_normal((16, D, HK), dtype=np.float32) * sd
    Wv = rng.standard_normal((16, D, HK), dtype=np.float32) * sd
    Wo = rng.standard_normal((D, D), dtype=np.float32) * sd
    bo = rng.standard_normal((D,), dtype=np.float32) * 0.01
    o = kernel(q, k, v, Wq, Wk, Wv, Wo, bo)
    print("out", o.shape, o.dtype, np.abs(o).max())


# revision 10
# speedup vs baseline: 3.4335x; 1.0376x over previous
"""MultiHeadAttention Trainium2 kernel.

Sharding: 8 cores = 4 batches x 2 head-halves. Core c handles batch c//2,
heads (c%2)*8 .. +8. I/O is minimized (the axon per-call cost is dominated
by host->device bytes): each core receives ONE packed bf16 input tensor
holding only its seq-half of X^T (q/k/v, pre-transposed on host) plus a
quarter shard of its head-half weights. On device, a pair AllGather
rebuilds full X^T per batch, a 4-way AllGather rebuilds the weights, and
a pair ReduceScatter sums the two head-half output projections so each
core returns only half the batch output rows (bf16).

Packed input xin [3584, 1024] per core (b = c//2, half = c%2):
  rows    0:1024  q^T of batch b, seq rows half*1024..+1024  [d, s_half]
  rows 1024:2048  k^T   (same slice)
  rows 2048:3072  v^T   (same slice)
  rows 3072:3584  weight quarter pack: for each of (wq, wk, wv, wot) in
                  [128 part, 4096] layout, partitions 32b..32b+32,
                  re-flowed to 128 rows of 1024.

Per-core dataflow (S=2048 seq, D=1024 model, 8 local heads of 64):
  phase 1: project q^T (zero-padded per head to 128 partitions for
           head-pair packed score matmuls), k^T (head-pair layout), and
           v_aug (v chunks with a ones column -> softmax denominator
           comes free out of the ctx matmul). All matmuls bf16.
  phase 2: per (q-tile of 512, head): scores^T = k^T_pair.T @ q_pad
           (PSUM f32), exp on ACT straight PSUM->SBUF bf16 (scale
           1/sqrt(D) folded in), ctx^T[65,512] = [v|1]^T @ E accumulated
           over 16 key chunks; row 64 is the softmax denominator.
           Normalize via 1/d = exp(-ln(d)) on ACT, broadcast across
           partitions with a ones[1,128] PE matmul, one DVE multiply per
           head. Output projection per q-tile -> partial out (bf16) ->
           ReduceScatter(add) over the batch pair.
"""

import os
import sys

# The bass kernel needs the TRN (axon) jax backend; if the caller pinned
# JAX_PLATFORMS=cpu for its reference computation, undo that before jax
# gets imported (no-op if jax is already initialized).
if os.environ.get("JAX_PLATFORMS") == "cpu":
    del os.environ["JAX_PLATFORMS"]

sys.path.insert(0, "/opt/trn_rl_repo")

import numpy as np
import ml_dtypes

import concourse.bass as bass
import concourse.mybir as mybir
import concourse.tile as tile
from concourse import bacc
from concourse.bass_utils import run_bass_kernel_spmd
import concourse.hw_specs as hw_specs

_orig_get_act_tables = hw_specs.get_activation_tables


def _only_ln_exp_set(module_arch):
    # Keep all set names in original order (set_id = dict index), but leave
    # only natural_log_exp_and_others non-empty so every activation resolves
    # to that one set -> a single ACT_TABLE_LOAD for exp+ln+copy.
    t = _orig_get_act_tables(module_arch)
    name = "natural_log_exp_and_others"
    assert name in t, list(t)
    return {k: (v if k == name else set()) for k, v in t.items()}


hw_specs.get_activation_tables = _only_ln_exp_set
bacc.get_activation_tables = _only_ln_exp_set

f32 = mybir.dt.float32
bf16 = mybir.dt.bfloat16
P = 128
S = 2048
D = 1024
SH = S // 2       # per-core seq half
HLOC = 8          # heads per core
HK = 64           # head size
DKL = HLOC * HK   # local projected dims = 512
QT = 512          # q-tile size
NQT = S // QT     # 4
NKC = S // P      # 16 key chunks
XROWS = 3 * D     # 3072 packed X^T rows
WROWS = 512       # packed weight-quarter rows
EXP = mybir.ActivationFunctionType.Exp
LN = mybir.ActivationFunctionType.Ln
SCALE = 1.0 / np.sqrt(np.float32(D))  # = 1/32, exact

PAIRS = [[0, 1], [2, 3], [4, 5], [6, 7]]
QUADS = [[0, 2, 4, 6], [1, 3, 5, 7]]


def build_nc():
    nc = bacc.Bacc("TRN2", target_bir_lowering=False)
    xin = nc.dram_tensor("xin", [XROWS + WROWS, SH], bf16, kind="ExternalInput")
    out = nc.dram_tensor("out", [SH, D], bf16, kind="ExternalOutput")

    ds = bass.ds

    with tile.TileContext(nc) as tc:
        with tc.tile_pool(name="dram", bufs=1, space="DRAM") as dram, tc.tile_pool(
            name="persist", bufs=1
        ) as pp:
            # collective bounce/result buffers (collectives can't touch I/O
            # tensors). Per-tensor AllGathers so q projections start after
            # only q's 2MB has crossed the pair link, k/v gather in the
            # background.
            xbq = dram.tile([D, SH], bf16, name="xbq")
            xbk = dram.tile([D, SH], bf16, name="xbk")
            xbv = dram.tile([D, SH], bf16, name="xbv")
            wb = dram.tile([WROWS, SH], bf16, name="wb")
            xgq = dram.tile([2, D, SH], bf16, name="xgq")
            xgk = dram.tile([2, D, SH], bf16, name="xgk")
            xgv = dram.tile([2, D, SH], bf16, name="xgv")
            wg = dram.tile([512, 4096], bf16, name="wg")
            po0 = dram.tile([SH, D], bf16, name="po0")
            po1 = dram.tile([SH, D], bf16, name="po1")
            og0 = dram.tile([SH // 2, D], bf16, name="og0")
            og1 = dram.tile([SH // 2, D], bf16, name="og1")

            nc.gpsimd.dma_start(wb[:], xin[ds(XROWS, WROWS), :])
            for t, xbt in enumerate((xbq, xbk, xbv)):
                nc.gpsimd.dma_start(xbt[:], xin[ds(t * D, D), :])
            nc.gpsimd.collective_compute(
                "AllGather",
                mybir.AluOpType.bypass,
                replica_groups=QUADS,
                ins=[wb.opt()],
                outs=[wg.opt()],
            )
            for xbt, xgt in ((xbq, xgq), (xbk, xgk), (xbv, xgv)):
                nc.gpsimd.collective_compute(
                    "AllGather",
                    mybir.AluOpType.bypass,
                    replica_groups=PAIRS,
                    ins=[xbt.opt()],
                    outs=[xgt.opt()],
                )

            # head h's q^T, zero-padded to the 128-partition pair layout
            qpad = pp.tile([P, HLOC, S], bf16, name="qpad")
            # k^T in head-pair layout: [:, p, :] rows 0-63 = head 2p, 64-127 = 2p+1
            kT = pp.tile([P, HLOC // 2, S], bf16, name="kT")
            # v chunks + ones column: [k-part, chunk, head, 64 v | 1]
            vaug = pp.tile([P, NKC, HLOC, HK + 1], bf16, name="vaug")
            ones1 = pp.tile([1, P], bf16, name="ones1")
            nc.vector.memset(ones1[:], 1.0)
            nc.vector.memset(qpad[:], 0.0)
            nc.vector.memset(vaug[:], 1.0)

            # weights in SBUF: [128 part, (dc|cc)*free] flattened layouts
            wqs = pp.tile([P, 4096], bf16, name="wqs")
            wks = pp.tile([P, 4096], bf16, name="wks")
            wvs = pp.tile([P, 4096], bf16, name="wvs")
            wos = pp.tile([P, 4096], bf16, name="wos")
            for k in range(4):
                for i, wt in enumerate((wqs, wks, wvs, wos)):
                    nc.sync.dma_start(
                        wt[ds(32 * k, 32), :], wg[ds(128 * k + 32 * i, 32), :]
                    )

            # ---------------- phase 1: projections (X^T arrives gathered) ----
            with tc.tile_pool(name="p1sb", bufs=1) as p1, tc.tile_pool(
                name="pr_ps", bufs=3, space="PSUM"
            ) as pr_ps:
                for t, (ws, xgt) in enumerate(
                    ((wqs, xgq), (wks, xgk), (wvs, xgv))
                ):
                    for st in range(S // QT):
                        r, sloc = st // 2, (st % 2) * QT
                        xt = p1.tile([P, D // P, QT], bf16, name="xt", tag="xt", bufs=3)
                        for dc in range(D // P):
                            nc.sync.dma_start(
                                xt[:, dc, :],
                                xgt[r, ds(dc * P, P), ds(sloc, QT)],
                            )
                        if t < 2:
                            # q^T / k^T orientation: psum [128 dk, 512 s]
                            for dkb in range(DKL // P):
                                ppr = pr_ps.tile([P, QT], f32, name="ppr")
                                for dc in range(D // P):
                                    nc.tensor.matmul(
                                        ppr[:],
                                        ws[:, ds(dc * DKL + dkb * P, P)],
                                        xt[:, dc, :],
                                        start=(dc == 0),
                                        stop=(dc == D // P - 1),
                                    )
                                if t == 0:
                                    nc.vector.tensor_copy(
                                        qpad[0:64, 2 * dkb, ds(st * QT, QT)],
                                        ppr[0:64, :],
                                    )
                                    nc.vector.tensor_copy(
                                        qpad[64:128, 2 * dkb + 1, ds(st * QT, QT)],
                                        ppr[64:128, :],
                                    )
                                else:
                                    nc.vector.tensor_copy(
                                        kT[:, dkb, ds(st * QT, QT)], ppr[:]
                                    )
                        else:
                            # v orientation: psum [128 s, 512 dk]
                            for sb in range(4):
                                ppr = pr_ps.tile([P, DKL], f32, name="ppr")
                                for dc in range(D // P):
                                    nc.tensor.matmul(
                                        ppr[:],
                                        xt[:, dc, ds(sb * P, P)],
                                        ws[:, ds(dc * DKL, DKL)],
                                        start=(dc == 0),
                                        stop=(dc == D // P - 1),
                                    )
                                ci = st * 4 + sb
                                nc.vector.tensor_copy(
                                    vaug[:, ci, :, 0:HK],
                                    ppr[:].rearrange("p (h k) -> p h k", k=HK),
                                )

            # ---------------- phase 2: attention + output projection ----------
            with tc.tile_pool(name="et_pool", bufs=3) as etp, tc.tile_pool(
                name="bc_pool", bufs=2
            ) as bcp, tc.tile_pool(name="cu_pool", bufs=1) as cup, tc.tile_pool(
                name="ct_pool", bufs=2
            ) as ctp, tc.tile_pool(
                name="ob_pool", bufs=2
            ) as obp, tc.tile_pool(
                name="sc_ps", bufs=2, space="PSUM"
            ) as sc_ps, tc.tile_pool(
                name="ct_ps", bufs=1, space="PSUM"
            ) as ct_ps, tc.tile_pool(
                name="op_ps", bufs=1, space="PSUM"
            ) as op_ps, tc.tile_pool(
                name="bc_ps", bufs=2, space="PSUM"
            ) as bc_ps:
                for qt in range(NQT):
                    ctxU = cup.tile([P, DKL // P, QT], f32, name="ctxU")
                    ctxT = ctp.tile([P, DKL // P, QT], bf16, name="ctxT")
                    for h in range(HLOC):
                        p = h // 2
                        pct = ct_ps.tile([HK + 1, QT], f32, name="pct")
                        for g in range(NKC // 2):
                            psc = sc_ps.tile([P, 2, QT], f32, name="psc")
                            for j in range(2):
                                ch = 2 * g + j
                                nc.tensor.matmul(
                                    psc[:, j, :],
                                    kT[:, p, ds(ch * P, P)],
                                    qpad[:, h, ds(qt * QT, QT)],
                                    start=True,
                                    stop=True,
                                )
                            ete = etp.tile([P, 2, QT], bf16, name="ete")
                            nc.scalar.activation(ete[:], psc[:], EXP, scale=SCALE)
                            for j in range(2):
                                ch = 2 * g + j
                                nc.tensor.matmul(
                                    pct[:],
                                    vaug[:, ch, h, :],
                                    ete[:, j, :],
                                    start=(ch == 0),
                                    stop=(ch == NKC - 1),
                                )
                        sl = slice(64 * (h % 2), 64 * (h % 2) + 64)
                        nc.vector.tensor_copy(ctxU[sl, h // 2, :], pct[0:64, :])
                        dln = bcp.tile([1, QT], f32, name="dln", tag="dln", bufs=3)
                        nc.scalar.activation(dln[:], pct[64:65, :], LN)
                        drec = bcp.tile([1, QT], bf16, name="drec", tag="drec", bufs=3)
                        nc.scalar.activation(drec[:], dln[:], EXP, scale=-1.0)
                        bch = bc_ps.tile([P, QT], f32, name="bch")
                        nc.tensor.matmul(bch[:], ones1[:], drec[:], start=True, stop=True)
                        nc.vector.tensor_mul(
                            out=ctxT[sl, h // 2, :],
                            in0=ctxU[sl, h // 2, :],
                            in1=bch[sl, :],
                        )
                    for sb in range(QT // P):
                        ob = obp.tile([P, 2, QT], bf16, name="ob")
                        for nh in range(2):
                            pop = op_ps.tile([P, QT], f32, name="pop")
                            for cc in range(DKL // P):
                                nc.tensor.matmul(
                                    pop[:],
                                    ctxT[:, cc, ds(sb * P, P)],
                                    wos[:, ds(cc * D + nh * QT, QT)],
                                    start=(cc == 0),
                                    stop=(cc == DKL // P - 1),
                                )
                            nc.vector.tensor_copy(ob[:, nh, :], pop[:])
                        pot = po0 if qt < 2 else po1
                        nc.sync.dma_start(
                            pot[ds((qt % 2) * QT + sb * P, P), :],
                            ob[:].rearrange("p a b -> p (a b)"),
                        )
                    if qt == 1:
                        # first output half reduces while qt 2,3 compute
                        nc.gpsimd.collective_compute(
                            "ReduceScatter",
                            mybir.AluOpType.add,
                            replica_groups=PAIRS,
                            ins=[po0.opt()],
                            outs=[og0.opt()],
                        )
                        nc.gpsimd.dma_start(out[ds(0, SH // 2), :], og0[:])

            nc.gpsimd.collective_compute(
                "ReduceScatter",
                mybir.AluOpType.add,
                replica_groups=PAIRS,
                ins=[po1.opt()],
                outs=[og1.opt()],
            )
            nc.gpsimd.dma_start(out[ds(SH // 2, SH // 2), :], og1[:])
    nc.compile()
    return nc


_NC_CACHE = []


def _get_nc():
    if not _NC_CACHE:
        _NC_CACHE.append(build_nc())
    return _NC_CACHE[0]


def _pack_w(w_hh, quarter):
    # [1024, 512] -> [128 part, 4096] (p, dc*512+n) -> quarter [32, 4096]
    # re-flowed to [128, 1024] rows of the packed input.
    a = w_hh.reshape(8, 128, 512).transpose(1, 0, 2).reshape(128, 4096)
    return a[32 * quarter : 32 * quarter + 32].reshape(128, 1024)


def _pack_wot(wot_c, quarter):
    # [512, 1024] -> [128 part, 4096] (p, cc*1024+n) -> quarter as above
    a = wot_c.reshape(4, 128, 1024).transpose(1, 0, 2).reshape(128, 4096)
    return a[32 * quarter : 32 * quarter + 32].reshape(128, 1024)


def make_in_maps(query, key_in, value, Wq, Wk, Wv, Wo, bo):
    query = np.asarray(query, dtype=np.float32)
    key_in = np.asarray(key_in, dtype=np.float32)
    value = np.asarray(value, dtype=np.float32)
    Wq = np.asarray(Wq, dtype=np.float32)
    Wk = np.asarray(Wk, dtype=np.float32)
    Wv = np.asarray(Wv, dtype=np.float32)
    Wo = np.asarray(Wo, dtype=np.float32)
    in_maps = []
    for c in range(8):
        b, half = c // 2, c % 2
        h0 = half * HLOC
        xin = np.empty((XROWS + WROWS, SH), dtype=ml_dtypes.bfloat16)
        rows = slice(half * SH, (half + 1) * SH)
        xin[0:D] = query[b, rows, :].T
        xin[D : 2 * D] = key_in[b, rows, :].T
        xin[2 * D : 3 * D] = value[b, rows, :].T
        wq_hh = Wq[h0 : h0 + HLOC].transpose(1, 0, 2).reshape(D, DKL)
        wk_hh = Wk[h0 : h0 + HLOC].transpose(1, 0, 2).reshape(D, DKL)
        wv_hh = Wv[h0 : h0 + HLOC].transpose(1, 0, 2).reshape(D, DKL)
        wot_c = np.ascontiguousarray(Wo[:, h0 * HK : h0 * HK + DKL].T)
        xin[XROWS + 0 : XROWS + 128] = _pack_w(wq_hh, b)
        xin[XROWS + 128 : XROWS + 256] = _pack_w(wk_hh, b)
        xin[XROWS + 256 : XROWS + 384] = _pack_w(wv_hh, b)
        xin[XROWS + 384 : XROWS + 512] = _pack_wot(wot_c, b)
        in_maps.append({"xin": xin})
    return in_maps


def gather_out(results, B, bo):
    """results: list of 8 per-core dicts with 'out' [SH, D] bf16.

    Core 2b returns [rows 0:512 | rows 1024:1536], core 2b+1 returns
    [rows 512:1024 | rows 1536:2048] (split ReduceScatter halves).
    """
    bo = np.asarray(bo, dtype=np.float32)
    H2 = SH // 2
    out = np.empty((B, S, D), dtype=np.float32)
    for b in range(B):
        lo, hi = results[2 * b]["out"], results[2 * b + 1]["out"]
        out[b, 0:H2] = lo[0:H2]
        out[b, H2 : 2 * H2] = hi[0:H2]
        out[b, 2 * H2 : 3 * H2] = lo[H2:SH]
        out[b, 3 * H2 : 4 * H2] = hi[H2:SH]
    out += bo
    return out


def kernel(query, key_in, value, Wq, Wk, Wv, Wo, bo):
    nc = _get_nc()
    in_maps = make_in_maps(query, key_in, value, Wq, Wk, Wv, Wo, bo)
    res = run_bass_kernel_spmd(nc, in_maps, list(range(8)))
    return gather_out(res.results, np.asarray(query).shape[0], bo)


if __name__ == "__main__":
    rng = np.random.default_rng(0)
    q = rng.standard_normal((4, S, D), dtype=np.float32)
    k = rng.standard_normal((4, S, D), dtype=np.float32)
    v = rng.standard_normal((4, S, D), dtype=np.float32)
    sd = 1.0 / np.sqrt(D)
    Wq = rng.standard_normal((16, D, HK), dtype=np.float32) * sd
    Wk = rng.standard_normal((16, D, HK), dtype=np.float32) * sd
    Wv = rng.standard_normal((16, D, HK), dtype=np.float32) * sd
    Wo = rng.standard_normal((D, D), dtype=np.float32) * sd
    bo = rng.standard_normal((D,), dtype=np.float32) * 0.01
    o = kernel(q, k, v, Wq, Wk, Wv, Wo, bo)
    print("out", o.shape, o.dtype, np.abs(o).max())
